# revision 20
# baseline (speedup 1.0000x reference)
"""Multi-layer GCN (2x GCNConv + linear head) on 8 Trainium2 NeuronCores.

Strategy (graph/data parallel, node-sharded):
  - Nodes are partitioned contiguously across the 8 cores (6250 each).
  - Each core aggregates messages for its own dst nodes. Edges are bucketed
    by dst tile (128 dsts) on the host, sorted, and padded to 16-idx
    granularity.
  - Gather of source-node features uses dma_gather (batched indirect DMA)
    from a full node-feature table in HBM. Since dma_gather indices are
    int16, edges are split per tile into src<32768 and src>=32768 groups,
    the latter gathered from an offset view of the table.
  - Self-loop edges are not gathered at all: each tile's own rows are a
    contiguous HWDGE DMA load from the core's node-major shard, accumulated
    via a diagonal matmul built on the ScalarEngine.
  - The scatter-add (segment sum) runs on the TensorEngine: for each
    128-edge block, a one-hot scatter matrix S[e, d] = (dst_e == d) *
    deg_isqrt[dst_e] is built on the VectorEngine with a single
    tensor_scalar(is_equal, mult) against an iota row, and PSUM accumulates
    G_block.T @ S across blocks -> feature-major agg^T tile.
  - deg_isqrt[src] is pre-folded into the gather table rows, so the full
    GCN normalization D^-1/2 (A+I) D^-1/2 comes out of table-scale x S.
  - Layer weights are applied right on the feature-major agg tiles; layer-1
    output is transposed back to node-major (TensorE transpose), scaled by
    deg_isqrt (source-side fold for layer 2) and AllGathered so every core
    has the full h1 table for layer-2 gathers.
  - Layer-2 output stays feature-major and feeds the output projection
    directly (lhsT = h2^T), producing node-major [dst, 64] tiles.
"""

import os
import sys

sys.path.insert(0, "/opt/trn_rl_repo")

import numpy as np

N = int(os.environ.get("GCN_N", 50000))
C_IN = 128
HID = 128
C_OUT = 64
NCORES = 8
NPER = N // NCORES
P = 128
NT = (NPER + P - 1) // P
SPLIT = int(os.environ.get("GCN_SPLIT", 32768))  # int16 gather index limit

MAXIDX = 1024  # max idxs per dma_gather call (larger calls fault the device)

LAST_RESULT = None  # BassKernelResults of the most recent run (for test.py)


def _r16(n):
    return (int(n) + 15) // 16 * 16


def _preprocess(edge_index, x, W1, b1, W2, b2, Wo, bo):
    """Host-side graph preprocessing -> per-core input arrays + schedule."""
    src_e = np.asarray(edge_index[0], np.int64)
    dst_e = np.asarray(edge_index[1], np.int64)
    # degree includes the self loop
    deg = (np.bincount(dst_e, minlength=N) + 1).astype(np.float32)
    disqrt = (1.0 / np.sqrt(deg)).astype(np.float32)

    # gather table: x pre-scaled by src-side normalization
    xs = (np.asarray(x, np.float32) * disqrt[:, None]).astype(np.float16)

    # per (core, tile) edge buckets (no self loops)
    per_core = []
    nlo = np.zeros((NCORES, NT), np.int64)
    nhi = np.zeros((NCORES, NT), np.int64)
    for c in range(NCORES):
        m = (dst_e >= c * NPER) & (dst_e < (c + 1) * NPER)
        s_c = src_e[m]
        d_c = dst_e[m] - c * NPER
        order = np.argsort(d_c, kind="stable")
        s_c, d_c = s_c[order], d_c[order]
        bounds = np.searchsorted(d_c, np.arange(0, NT + 1) * P)
        tiles = []
        for t in range(NT):
            ss = s_c[bounds[t]:bounds[t + 1]]
            dd = d_c[bounds[t]:bounds[t + 1]] - t * P
            lo = ss < SPLIT
            tiles.append((ss[lo], dd[lo], ss[~lo], dd[~lo]))
            nlo[c, t] = lo.sum()
            nhi[c, t] = (~lo).sum()
        per_core.append(tiles)

    # padded idx counts per tile/group (uniform across cores), 16-granular
    M_lo = [_r16(nlo[:, t].max()) for t in range(NT)]
    M_hi = [_r16(nhi[:, t].max()) for t in range(NT)]
    # chunk counts (128-edge blocks fed to matmuls)
    B_lo = [(m + P - 1) // P for m in M_lo]
    B_hi = [(m + P - 1) // P for m in M_hi]
    NB = int(sum(B_lo) + sum(B_hi))
    NC16 = int(sum(M_lo) + sum(M_hi)) // 16  # idx columns (16 idx each)
    tile_ws = [min(P, NPER - t * P) for t in range(NT)]

    in_maps = []
    for c in range(NCORES):
        idx16 = np.zeros((16, NC16), np.int16)
        dstloc = np.zeros((P, NB), np.float32)
        dscale = np.zeros((P, NB), np.float32)
        col16 = 0
        blk = 0
        for t in range(NT):
            ss_lo, dd_lo, ss_hi, dd_hi = per_core[c][t]
            for (ss, dd, m_pad, base) in (
                (ss_lo, dd_lo, M_lo[t], 0),
                (ss_hi, dd_hi, M_hi[t], SPLIT),
            ):
                nb = (m_pad + P - 1) // P
                if nb == 0:
                    continue
                n = len(ss)
                flat_i = np.zeros(m_pad, np.int16)
                flat_i[:n] = (ss - base).astype(np.int16)
                idx16[:, col16:col16 + m_pad // 16] = \
                    flat_i.reshape(m_pad // 16, 16).T
                col16 += m_pad // 16
                flat_d = np.zeros(nb * P, np.float32)
                flat_s = np.zeros(nb * P, np.float32)
                flat_d[:n] = -dd.astype(np.float32)
                flat_s[:n] = disqrt[dd + t * P + c * NPER]
                dstloc[:, blk:blk + nb] = flat_d.reshape(nb, P).T
                dscale[:, blk:blk + nb] = flat_s.reshape(nb, P).T
                blk += nb
        assert col16 == NC16 and blk == NB
        idx_full = np.tile(idx16, (8, 1)).astype(np.int16)

        dsqnm = np.zeros((P, NT), np.float32)
        for t in range(NT):
            tw = tile_ws[t]
            dsqnm[:tw, t] = disqrt[c * NPER + t * P: c * NPER + t * P + tw]

        iota = np.tile(np.arange(P, dtype=np.float16)[None, :], (P, 1))

        in_maps.append({
            "xs": xs,
            "xss": xs[c * NPER:(c + 1) * NPER].copy(),
            "idx": idx_full,
            "dstloc": dstloc,
            "dscale": dscale,
            "ndscale": -dscale,
            "dsqnm": dsqnm,
            "iota": iota,
            "w1": np.asarray(W1, np.float32).astype(np.float16),
            "w2": np.asarray(W2, np.float32).astype(np.float16),
            "wo": np.asarray(Wo, np.float32).astype(np.float16),
            "b1": np.asarray(b1, np.float32).reshape(HID, 1).copy(),
            "b2": np.asarray(b2, np.float32).reshape(HID, 1).copy(),
            "bo": np.tile(np.asarray(bo, np.float32)[None, :], (P, 1)),
        })

    sched = dict(M_lo=M_lo, M_hi=M_hi, B_lo=B_lo, B_hi=B_hi, NB=NB,
                 NC16=NC16, tile_ws=tile_ws)
    return in_maps, sched


def _build_program(sched):
    import concourse.bass as bass
    import concourse.bacc as bacc
    import concourse.tile as tile
    import concourse.mybir as mybir
    from concourse.masks import make_identity

    f32 = mybir.dt.float32
    f16 = mybir.dt.float16
    i16 = mybir.dt.int16
    M_lo, M_hi = sched["M_lo"], sched["M_hi"]
    B_lo, B_hi = sched["B_lo"], sched["B_hi"]
    NB, NC16, tile_ws = sched["NB"], sched["NC16"], sched["tile_ws"]
    nblk_max = max(bl + bh for bl, bh in zip(B_lo, B_hi))

    nc = bacc.Bacc("TRN2", target_bir_lowering=False, debug=False,
                   num_devices=NCORES)

    xs_d = nc.dram_tensor("xs", [N, C_IN], f16, kind="ExternalInput")
    xss_d = nc.dram_tensor("xss", [NPER, C_IN], f16, kind="ExternalInput")
    idx_d = nc.dram_tensor("idx", [P, NC16], i16, kind="ExternalInput")
    dstloc_d = nc.dram_tensor("dstloc", [P, NB], f32, kind="ExternalInput")
    dscale_d = nc.dram_tensor("dscale", [P, NB], f32, kind="ExternalInput")
    ndscale_d = nc.dram_tensor("ndscale", [P, NB], f32, kind="ExternalInput")
    dsqnm_d = nc.dram_tensor("dsqnm", [P, NT], f32, kind="ExternalInput")
    iota_d = nc.dram_tensor("iota", [P, P], f16, kind="ExternalInput")
    w1_d = nc.dram_tensor("w1", [C_IN, HID], f16, kind="ExternalInput")
    w2_d = nc.dram_tensor("w2", [HID, HID], f16, kind="ExternalInput")
    wo_d = nc.dram_tensor("wo", [HID, C_OUT], f16, kind="ExternalInput")
    b1_d = nc.dram_tensor("b1", [HID, 1], f32, kind="ExternalInput")
    b2_d = nc.dram_tensor("b2", [HID, 1], f32, kind="ExternalInput")
    bo_d = nc.dram_tensor("bo", [P, C_OUT], f32, kind="ExternalInput")
    out_d = nc.dram_tensor("out", [NPER, C_OUT], f32, kind="ExternalOutput")

    with tile.TileContext(nc) as tc:
        with tc.tile_pool(name="const", bufs=1) as cpool, \
             tc.tile_pool(name="gather", bufs=4) as gpool, \
             tc.tile_pool(name="smat", bufs=10) as spool, \
             tc.tile_pool(name="work", bufs=3) as wpool, \
             tc.tile_pool(name="psA", bufs=2, space="PSUM") as psA, \
             tc.tile_pool(name="psH", bufs=2, space="PSUM") as psH, \
             tc.tile_pool(name="psT", bufs=2, space="PSUM") as psT, \
             tc.tile_pool(name="dram", bufs=1, space="DRAM") as dram:

            def cload(name, dram_t, shape, dt):
                t = cpool.tile(shape, dt, name=name)
                nc.sync.dma_start(t[:], dram_t[tuple(slice(0, s) for s in shape)])
                return t

            idx_sb = cload("idx_sb", idx_d, [P, NC16], i16)
            dstloc_sb = cload("dstloc_sb", dstloc_d, [P, NB], f32)
            dscale_sb = cload("dscale_sb", dscale_d, [P, NB], f32)
            ndscale_sb = cload("ndscale_sb", ndscale_d, [P, NB], f32)
            dsqnm_sb = cload("dsqnm_sb", dsqnm_d, [P, NT], f32)
            iota_sb = cload("iota_sb", iota_d, [P, P], f16)
            w1_sb = cload("w1_sb", w1_d, [C_IN, HID], f16)
            w2_sb = cload("w2_sb", w2_d, [HID, HID], f16)
            wo_sb = cload("wo_sb", wo_d, [HID, C_OUT], f16)
            b1_sb = cload("b1_sb", b1_d, [HID, 1], f32)
            b2_sb = cload("b2_sb", b2_d, [HID, 1], f32)
            bo_sb = cload("bo_sb", bo_d, [P, C_OUT], f32)

            ident_sb = cpool.tile([P, P], f16, name="ident_sb")
            make_identity(nc, ident_sb[:])

            h1s = dram.tile([NPER, HID], f16, name="h1s")
            h1f = dram.tile([N, HID], f16, name="h1f", addr_space="Shared")

            # register cache for num_idxs_reg values
            regs = {}

            def reg_of(v):
                if v not in regs:
                    regs[v] = nc.gpsimd.to_reg(v)
                return regs[v]

            def layer(phase):
                w_sb = w1_sb if phase == 0 else w2_sb
                b_sb = b1_sb if phase == 0 else b2_sb
                tbl = xs_d if phase == 0 else h1f
                shard = xss_d if phase == 0 else h1s
                col16 = 0
                blk = 0
                for t in range(NT):
                    tw = tile_ws[t]
                    blo, bhi = B_lo[t], B_hi[t]
                    nblk = blo + bhi
                    G = gpool.tile([P, nblk_max, C_IN], f16, tag="G", name="G")
                    for (m_pad, goff, base) in ((M_lo[t], 0, 0),
                                                (M_hi[t], blo, SPLIT)):
                        if m_pad == 0:
                            continue
                        src = tbl[base:, :] if base else tbl[:, :]
                        for o in range(0, m_pad, MAXIDX):
                            n_call = min(MAXIDX, m_pad - o)
                            c0 = col16 + o // 16
                            ob = goff + o // P
                            nc.gpsimd.dma_gather(
                                out_ap=G[:, ob:ob + (n_call + P - 1) // P, :],
                                in_ap=src,
                                idxs_ap=idx_sb[:, c0:c0 + (n_call + 15) // 16],
                                num_idxs=n_call,
                                num_idxs_reg=reg_of(n_call),
                                elem_size=C_IN)
                        col16 += m_pad // 16
                    pa = psA.tile([P, tw], f32, tag="pa", name="pa")
                    # self-loop contribution: contiguous slab + diagonal matmul
                    slab = wpool.tile([P, C_IN], f16, tag="slab", name="slab")
                    nc.sync.dma_start(slab[:tw, :],
                                      shard[t * P:t * P + tw, :])
                    diag = spool.tile([P, P], f16, tag="S", name="diag")
                    nc.scalar.activation(diag[:, :tw], ident_sb[:, :tw],
                                         mybir.ActivationFunctionType.Copy,
                                         scale=dsqnm_sb[:, t:t + 1])
                    nc.tensor.matmul(pa[:], lhsT=slab[:tw, :],
                                     rhs=diag[:tw, :tw], start=True,
                                     stop=False)
                    # valid contraction rows per block (final block of each
                    # group is 16-granular; stale tail rows are never read)
                    ks = []
                    for (m_pad, nb) in ((M_lo[t], blo), (M_hi[t], bhi)):
                        if nb:
                            ks += [P] * (nb - 1) + [m_pad - (nb - 1) * P]
                    for j in range(nblk):
                        # S on ScalarE (keeps DVE off the shared GpSimd SBUF
                        # port): S = relu(dscale - dscale*(iota - dstloc)^2)
                        S1 = spool.tile([P, P], f16, tag="S", name="S1")
                        nc.scalar.activation(
                            S1[:, :tw], iota_sb[:, :tw],
                            mybir.ActivationFunctionType.Square,
                            bias=dstloc_sb[:, blk + j:blk + j + 1])
                        S = spool.tile([P, P], f16, tag="S", name="S")
                        nc.scalar.activation(
                            S[:, :tw], S1[:, :tw],
                            mybir.ActivationFunctionType.Relu,
                            scale=ndscale_sb[:, blk + j:blk + j + 1],
                            bias=dscale_sb[:, blk + j:blk + j + 1])
                        nc.tensor.matmul(pa[:], lhsT=G[:ks[j], j, :],
                                         rhs=S[:ks[j], :tw],
                                         start=False, stop=(j == nblk - 1))
                    blk += nblk
                    agg = wpool.tile([P, tw], f16, tag="agg", name="agg")
                    nc.vector.tensor_copy(agg[:], pa[:])
                    ph = psH.tile([P, tw], f32, tag="ph", name="ph")
                    nc.tensor.matmul(ph[:], lhsT=w_sb[:], rhs=agg[:],
                                     start=True, stop=True)
                    h = wpool.tile([P, tw], f16, tag="h", name="h")
                    nc.scalar.activation(h[:], ph[:],
                                         mybir.ActivationFunctionType.Relu,
                                         bias=b_sb[:, 0:1])
                    if phase == 0:
                        pt = psT.tile([P, P], f16, tag="pt", name="pt")
                        nc.tensor.transpose(out=pt[:tw, :], in_=h[:, :tw],
                                            identity=ident_sb[:])
                        hn = wpool.tile([P, P], f16, tag="hn", name="hn")
                        nc.vector.tensor_scalar(
                            out=hn[:tw, :], in0=pt[:tw, :],
                            scalar1=dsqnm_sb[:tw, t:t + 1], scalar2=None,
                            op0=mybir.AluOpType.mult)
                        nc.sync.dma_start(h1s[t * P:t * P + tw, :], hn[:tw, :])
                    else:
                        po = psT.tile([P, C_OUT], f32, tag="po", name="po")
                        nc.tensor.matmul(po[:tw, :], lhsT=h[:, :tw],
                                         rhs=wo_sb[:], start=True, stop=True)
                        ob = wpool.tile([P, C_OUT], f32, tag="ob", name="ob")
                        nc.vector.tensor_tensor(out=ob[:tw, :], in0=po[:tw, :],
                                                in1=bo_sb[:tw, :],
                                                op=mybir.AluOpType.add)
                        nc.sync.dma_start(out_d[t * P:t * P + tw, :],
                                          ob[:tw, :])

            layer(0)
            nc.gpsimd.collective_compute(
                "AllGather", mybir.AluOpType.bypass,
                replica_groups=[list(range(NCORES))],
                ins=[h1s[:].opt()], outs=[h1f[:].opt()])
            layer(1)

    nc.compile()
    return nc


def kernel(x, edge_index, W1, b1, W2, b2, Wo, bo):
    global LAST_RESULT
    from concourse import bass_utils

    in_maps, sched = _preprocess(edge_index, x, W1, b1, W2, b2, Wo, bo)
    nc = _build_program(sched)
    res = bass_utils.run_bass_kernel_spmd(nc, in_maps,
                                          core_ids=list(range(NCORES)))
    LAST_RESULT = res
    out = np.concatenate([res.results[c]["out"] for c in range(NCORES)], axis=0)
    return out.astype(np.float32)


# revision 21
# speedup vs baseline: 1.0626x; 1.0626x over previous
"""Multi-layer GCN (2x GCNConv + linear head) on 8 Trainium2 NeuronCores.

Strategy (graph/data parallel, node-sharded):
  - Nodes are partitioned contiguously across the 8 cores (6250 each).
  - Each core aggregates messages for its own dst nodes. Edges are bucketed
    by dst tile (128 dsts) on the host, sorted, and padded to 16-idx
    granularity.
  - Gather of source-node features uses dma_gather (batched indirect DMA)
    from a full node-feature table in HBM. Since dma_gather indices are
    int16, edges are split per tile into src<32768 and src>=32768 groups,
    the latter gathered from an offset view of the table.
  - Self-loop edges are not gathered at all: each tile's own rows are a
    contiguous HWDGE DMA load from the core's node-major shard, accumulated
    via a diagonal matmul built on the ScalarEngine.
  - The scatter-add (segment sum) runs on the TensorEngine: for each
    128-edge block, a one-hot scatter matrix S[e, d] = (dst_e == d) *
    deg_isqrt[dst_e] is built on the VectorEngine with a single
    tensor_scalar(is_equal, mult) against an iota row, and PSUM accumulates
    G_block.T @ S across blocks -> feature-major agg^T tile.
  - deg_isqrt[src] is pre-folded into the gather table rows, so the full
    GCN normalization D^-1/2 (A+I) D^-1/2 comes out of table-scale x S.
  - Layer weights are applied right on the feature-major agg tiles; layer-1
    output is transposed back to node-major (TensorE transpose), scaled by
    deg_isqrt (source-side fold for layer 2) and AllGathered so every core
    has the full h1 table for layer-2 gathers.
  - Layer-2 output stays feature-major and feeds the output projection
    directly (lhsT = h2^T), producing node-major [dst, 64] tiles.
"""

import os
import sys

sys.path.insert(0, "/opt/trn_rl_repo")

import numpy as np

N = int(os.environ.get("GCN_N", 50000))
C_IN = 128
HID = 128
C_OUT = 64
NCORES = 8
NPER = N // NCORES
P = 128
NT = (NPER + P - 1) // P
SPLIT = int(os.environ.get("GCN_SPLIT", 25000))  # table split (int16 idx limit, balanced groups)

MAXIDX = 1024  # max idxs per dma_gather call (larger calls fault the device)

LAST_RESULT = None  # BassKernelResults of the most recent run (for test.py)


def _r16(n):
    return (int(n) + 15) // 16 * 16


def _preprocess(edge_index, x, W1, b1, W2, b2, Wo, bo):
    """Host-side graph preprocessing -> per-core input arrays + schedule."""
    src_e = np.asarray(edge_index[0], np.int64)
    dst_e = np.asarray(edge_index[1], np.int64)
    # degree includes the self loop
    deg = (np.bincount(dst_e, minlength=N) + 1).astype(np.float32)
    disqrt = (1.0 / np.sqrt(deg)).astype(np.float32)

    # gather table: x pre-scaled by src-side normalization
    xs = (np.asarray(x, np.float32) * disqrt[:, None]).astype(np.float16)

    # per (core, tile) edge buckets (no self loops)
    per_core = []
    nlo = np.zeros((NCORES, NT), np.int64)
    nhi = np.zeros((NCORES, NT), np.int64)
    for c in range(NCORES):
        m = (dst_e >= c * NPER) & (dst_e < (c + 1) * NPER)
        s_c = src_e[m]
        d_c = dst_e[m] - c * NPER
        order = np.argsort(d_c, kind="stable")
        s_c, d_c = s_c[order], d_c[order]
        bounds = np.searchsorted(d_c, np.arange(0, NT + 1) * P)
        tiles = []
        for t in range(NT):
            ss = s_c[bounds[t]:bounds[t + 1]]
            dd = d_c[bounds[t]:bounds[t + 1]] - t * P
            lo = ss < SPLIT
            tiles.append((ss[lo], dd[lo], ss[~lo], dd[~lo]))
            nlo[c, t] = lo.sum()
            nhi[c, t] = (~lo).sum()
        per_core.append(tiles)

    # padded idx counts per tile/group (uniform across cores), 16-granular
    M_lo = [_r16(nlo[:, t].max()) for t in range(NT)]
    M_hi = [_r16(nhi[:, t].max()) for t in range(NT)]
    # chunk counts (128-edge blocks fed to matmuls)
    B_lo = [(m + P - 1) // P for m in M_lo]
    B_hi = [(m + P - 1) // P for m in M_hi]
    NB = int(sum(B_lo) + sum(B_hi))
    NC16 = int(sum(M_lo) + sum(M_hi)) // 16  # idx columns (16 idx each)
    tile_ws = [min(P, NPER - t * P) for t in range(NT)]

    in_maps = []
    for c in range(NCORES):
        idx16 = np.zeros((16, NC16), np.int16)
        dstloc = np.zeros((P, NB), np.float32)
        dscale = np.zeros((P, NB), np.float32)
        col16 = 0
        blk = 0
        for t in range(NT):
            ss_lo, dd_lo, ss_hi, dd_hi = per_core[c][t]
            for (ss, dd, m_pad, base) in (
                (ss_lo, dd_lo, M_lo[t], 0),
                (ss_hi, dd_hi, M_hi[t], SPLIT),
            ):
                nb = (m_pad + P - 1) // P
                if nb == 0:
                    continue
                n = len(ss)
                flat_i = np.zeros(m_pad, np.int16)
                flat_i[:n] = (ss - base).astype(np.int16)
                idx16[:, col16:col16 + m_pad // 16] = \
                    flat_i.reshape(m_pad // 16, 16).T
                col16 += m_pad // 16
                flat_d = np.zeros(nb * P, np.float32)
                flat_s = np.zeros(nb * P, np.float32)
                flat_d[:n] = -dd.astype(np.float32)
                flat_s[:n] = disqrt[dd + t * P + c * NPER]
                dstloc[:, blk:blk + nb] = flat_d.reshape(nb, P).T
                dscale[:, blk:blk + nb] = flat_s.reshape(nb, P).T
                blk += nb
        assert col16 == NC16 and blk == NB
        idx_full = np.tile(idx16, (8, 1)).astype(np.int16)

        dsqnm = np.zeros((P, NT), np.float32)
        for t in range(NT):
            tw = tile_ws[t]
            dsqnm[:tw, t] = disqrt[c * NPER + t * P: c * NPER + t * P + tw]

        iota = np.tile(np.arange(P, dtype=np.float16)[None, :], (P, 1))

        in_maps.append({
            "xs": xs,
            "xss": xs[c * NPER:(c + 1) * NPER].copy(),
            "idx": idx_full,
            "dstloc": dstloc,
            "dscale": dscale,
            "ndscale": -dscale,
            "dsqnm": dsqnm,
            "iota": iota,
            "w1": np.asarray(W1, np.float32).astype(np.float16),
            "w2": np.asarray(W2, np.float32).astype(np.float16),
            "wo": np.asarray(Wo, np.float32).astype(np.float16),
            "b1": np.asarray(b1, np.float32).reshape(HID, 1).copy(),
            "b2": np.asarray(b2, np.float32).reshape(HID, 1).copy(),
            "bo": np.tile(np.asarray(bo, np.float32)[None, :], (P, 1)),
        })

    sched = dict(M_lo=M_lo, M_hi=M_hi, B_lo=B_lo, B_hi=B_hi, NB=NB,
                 NC16=NC16, tile_ws=tile_ws)
    return in_maps, sched


def _build_program(sched):
    import concourse.bass as bass
    import concourse.bacc as bacc
    import concourse.tile as tile
    import concourse.mybir as mybir
    from concourse.masks import make_identity

    f32 = mybir.dt.float32
    f16 = mybir.dt.float16
    i16 = mybir.dt.int16
    M_lo, M_hi = sched["M_lo"], sched["M_hi"]
    B_lo, B_hi = sched["B_lo"], sched["B_hi"]
    NB, NC16, tile_ws = sched["NB"], sched["NC16"], sched["tile_ws"]
    nblk_max = max(bl + bh for bl, bh in zip(B_lo, B_hi))

    nc = bacc.Bacc("TRN2", target_bir_lowering=False, debug=False,
                   num_devices=NCORES)

    xs_d = nc.dram_tensor("xs", [N, C_IN], f16, kind="ExternalInput")
    xss_d = nc.dram_tensor("xss", [NPER, C_IN], f16, kind="ExternalInput")
    idx_d = nc.dram_tensor("idx", [P, NC16], i16, kind="ExternalInput")
    dstloc_d = nc.dram_tensor("dstloc", [P, NB], f32, kind="ExternalInput")
    dscale_d = nc.dram_tensor("dscale", [P, NB], f32, kind="ExternalInput")
    ndscale_d = nc.dram_tensor("ndscale", [P, NB], f32, kind="ExternalInput")
    dsqnm_d = nc.dram_tensor("dsqnm", [P, NT], f32, kind="ExternalInput")
    iota_d = nc.dram_tensor("iota", [P, P], f16, kind="ExternalInput")
    w1_d = nc.dram_tensor("w1", [C_IN, HID], f16, kind="ExternalInput")
    w2_d = nc.dram_tensor("w2", [HID, HID], f16, kind="ExternalInput")
    wo_d = nc.dram_tensor("wo", [HID, C_OUT], f16, kind="ExternalInput")
    b1_d = nc.dram_tensor("b1", [HID, 1], f32, kind="ExternalInput")
    b2_d = nc.dram_tensor("b2", [HID, 1], f32, kind="ExternalInput")
    bo_d = nc.dram_tensor("bo", [P, C_OUT], f32, kind="ExternalInput")
    out_d = nc.dram_tensor("out", [NPER, C_OUT], f32, kind="ExternalOutput")

    with tile.TileContext(nc) as tc:
        with tc.tile_pool(name="const", bufs=1) as cpool, \
             tc.tile_pool(name="gather", bufs=4) as gpool, \
             tc.tile_pool(name="smat", bufs=10) as spool, \
             tc.tile_pool(name="work", bufs=3) as wpool, \
             tc.tile_pool(name="psA", bufs=2, space="PSUM") as psA, \
             tc.tile_pool(name="psH", bufs=2, space="PSUM") as psH, \
             tc.tile_pool(name="psT", bufs=2, space="PSUM") as psT, \
             tc.tile_pool(name="dram", bufs=1, space="DRAM") as dram:

            def cload(name, dram_t, shape, dt):
                t = cpool.tile(shape, dt, name=name)
                nc.sync.dma_start(t[:], dram_t[tuple(slice(0, s) for s in shape)])
                return t

            idx_sb = cload("idx_sb", idx_d, [P, NC16], i16)
            dstloc_sb = cload("dstloc_sb", dstloc_d, [P, NB], f32)
            dscale_sb = cload("dscale_sb", dscale_d, [P, NB], f32)
            ndscale_sb = cload("ndscale_sb", ndscale_d, [P, NB], f32)
            dsqnm_sb = cload("dsqnm_sb", dsqnm_d, [P, NT], f32)
            iota_sb = cload("iota_sb", iota_d, [P, P], f16)
            w1_sb = cload("w1_sb", w1_d, [C_IN, HID], f16)
            w2_sb = cload("w2_sb", w2_d, [HID, HID], f16)
            wo_sb = cload("wo_sb", wo_d, [HID, C_OUT], f16)
            b1_sb = cload("b1_sb", b1_d, [HID, 1], f32)
            b2_sb = cload("b2_sb", b2_d, [HID, 1], f32)
            bo_sb = cload("bo_sb", bo_d, [P, C_OUT], f32)

            ident_sb = cpool.tile([P, P], f16, name="ident_sb")
            make_identity(nc, ident_sb[:])

            h1s = dram.tile([NPER, HID], f16, name="h1s")
            h1f = dram.tile([N, HID], f16, name="h1f", addr_space="Shared")

            # register cache for num_idxs_reg values
            regs = {}

            def reg_of(v):
                if v not in regs:
                    regs[v] = nc.gpsimd.to_reg(v)
                return regs[v]

            def layer(phase):
                w_sb = w1_sb if phase == 0 else w2_sb
                b_sb = b1_sb if phase == 0 else b2_sb
                tbl = xs_d if phase == 0 else h1f
                shard = xss_d if phase == 0 else h1s
                col16 = 0
                blk = 0
                for t in range(NT):
                    tw = tile_ws[t]
                    blo, bhi = B_lo[t], B_hi[t]
                    nblk = blo + bhi
                    G = gpool.tile([P, nblk_max, C_IN], f16, tag="G", name="G")
                    for (m_pad, goff, base) in ((M_lo[t], 0, 0),
                                                (M_hi[t], blo, SPLIT)):
                        if m_pad == 0:
                            continue
                        src = tbl[base:, :] if base else tbl[:, :]
                        for o in range(0, m_pad, MAXIDX):
                            n_call = min(MAXIDX, m_pad - o)
                            c0 = col16 + o // 16
                            ob = goff + o // P
                            nc.gpsimd.dma_gather(
                                out_ap=G[:, ob:ob + (n_call + P - 1) // P, :],
                                in_ap=src,
                                idxs_ap=idx_sb[:, c0:c0 + (n_call + 15) // 16],
                                num_idxs=n_call,
                                num_idxs_reg=reg_of(n_call),
                                elem_size=C_IN)
                        col16 += m_pad // 16
                    pa = psA.tile([P, tw], f32, tag="pa", name="pa")
                    # self-loop contribution: contiguous slab + diagonal matmul
                    slab = wpool.tile([P, C_IN], f16, tag="slab", name="slab")
                    nc.sync.dma_start(slab[:tw, :],
                                      shard[t * P:t * P + tw, :])
                    diag = spool.tile([P, P], f16, tag="S", name="diag")
                    nc.scalar.activation(diag[:, :tw], ident_sb[:, :tw],
                                         mybir.ActivationFunctionType.Copy,
                                         scale=dsqnm_sb[:, t:t + 1])
                    nc.tensor.matmul(pa[:], lhsT=slab[:tw, :],
                                     rhs=diag[:tw, :tw], start=True,
                                     stop=False)
                    # valid contraction rows per block (final block of each
                    # group is 16-granular; stale tail rows are never read)
                    ks = []
                    for (m_pad, nb) in ((M_lo[t], blo), (M_hi[t], bhi)):
                        if nb:
                            ks += [P] * (nb - 1) + [m_pad - (nb - 1) * P]
                    for j in range(nblk):
                        # S on ScalarE (keeps DVE off the shared GpSimd SBUF
                        # port): S = relu(dscale - dscale*(iota - dstloc)^2)
                        S1 = spool.tile([P, P], f16, tag="S", name="S1")
                        nc.scalar.activation(
                            S1[:, :tw], iota_sb[:, :tw],
                            mybir.ActivationFunctionType.Square,
                            bias=dstloc_sb[:, blk + j:blk + j + 1])
                        S = spool.tile([P, P], f16, tag="S", name="S")
                        nc.scalar.activation(
                            S[:, :tw], S1[:, :tw],
                            mybir.ActivationFunctionType.Relu,
                            scale=ndscale_sb[:, blk + j:blk + j + 1],
                            bias=dscale_sb[:, blk + j:blk + j + 1])
                        nc.tensor.matmul(pa[:], lhsT=G[:ks[j], j, :],
                                         rhs=S[:ks[j], :tw],
                                         start=False, stop=(j == nblk - 1))
                    blk += nblk
                    agg = wpool.tile([P, tw], f16, tag="agg", name="agg")
                    nc.vector.tensor_copy(agg[:], pa[:])
                    ph = psH.tile([P, tw], f32, tag="ph", name="ph")
                    nc.tensor.matmul(ph[:], lhsT=w_sb[:], rhs=agg[:],
                                     start=True, stop=True)
                    h = wpool.tile([P, tw], f16, tag="h", name="h")
                    nc.scalar.activation(h[:], ph[:],
                                         mybir.ActivationFunctionType.Relu,
                                         bias=b_sb[:, 0:1])
                    if phase == 0:
                        pt = psT.tile([P, P], f16, tag="pt", name="pt")
                        nc.tensor.transpose(out=pt[:tw, :], in_=h[:, :tw],
                                            identity=ident_sb[:])
                        hn = wpool.tile([P, P], f16, tag="hn", name="hn")
                        nc.vector.tensor_scalar(
                            out=hn[:tw, :], in0=pt[:tw, :],
                            scalar1=dsqnm_sb[:tw, t:t + 1], scalar2=None,
                            op0=mybir.AluOpType.mult)
                        nc.sync.dma_start(h1s[t * P:t * P + tw, :], hn[:tw, :])
                    else:
                        po = psT.tile([P, C_OUT], f32, tag="po", name="po")
                        nc.tensor.matmul(po[:tw, :], lhsT=h[:, :tw],
                                         rhs=wo_sb[:], start=True, stop=True)
                        ob = wpool.tile([P, C_OUT], f32, tag="ob", name="ob")
                        nc.vector.tensor_tensor(out=ob[:tw, :], in0=po[:tw, :],
                                                in1=bo_sb[:tw, :],
                                                op=mybir.AluOpType.add)
                        nc.sync.dma_start(out_d[t * P:t * P + tw, :],
                                          ob[:tw, :])

            layer(0)
            nc.gpsimd.collective_compute(
                "AllGather", mybir.AluOpType.bypass,
                replica_groups=[list(range(NCORES))],
                ins=[h1s[:].opt()], outs=[h1f[:].opt()])
            layer(1)

    nc.compile()
    return nc


def kernel(x, edge_index, W1, b1, W2, b2, Wo, bo):
    global LAST_RESULT
    from concourse import bass_utils

    in_maps, sched = _preprocess(edge_index, x, W1, b1, W2, b2, Wo, bo)
    nc = _build_program(sched)
    res = bass_utils.run_bass_kernel_spmd(nc, in_maps,
                                          core_ids=list(range(NCORES)))
    LAST_RESULT = res
    out = np.concatenate([res.results[c]["out"] for c in range(NCORES)], axis=0)
    return out.astype(np.float32)


# revision 22
# speedup vs baseline: 1.0674x; 1.0045x over previous
"""Multi-layer GCN (2x GCNConv + linear head) on 8 Trainium2 NeuronCores.

Strategy (graph/data parallel, node-sharded):
  - Nodes are partitioned contiguously across the 8 cores (6250 each).
  - Each core aggregates messages for its own dst nodes. Edges are bucketed
    by dst tile (128 dsts) on the host, sorted, and padded to 16-idx
    granularity.
  - Gather of source-node features uses dma_gather (batched indirect DMA)
    from a full node-feature table in HBM. Since dma_gather indices are
    int16, edges are split per tile into src<32768 and src>=32768 groups,
    the latter gathered from an offset view of the table.
  - Self-loop edges are not gathered at all: each tile's own rows are a
    contiguous HWDGE DMA load from the core's node-major shard, accumulated
    via a diagonal matmul built on the ScalarEngine.
  - The scatter-add (segment sum) runs on the TensorEngine: for each
    128-edge block, a one-hot scatter matrix S[e, d] = (dst_e == d) *
    deg_isqrt[dst_e] is built on the VectorEngine with a single
    tensor_scalar(is_equal, mult) against an iota row, and PSUM accumulates
    G_block.T @ S across blocks -> feature-major agg^T tile.
  - deg_isqrt[src] is pre-folded into the gather table rows, so the full
    GCN normalization D^-1/2 (A+I) D^-1/2 comes out of table-scale x S.
  - Layer weights are applied right on the feature-major agg tiles; layer-1
    output is transposed back to node-major (TensorE transpose), scaled by
    deg_isqrt (source-side fold for layer 2) and AllGathered so every core
    has the full h1 table for layer-2 gathers.
  - Layer-2 output stays feature-major and feeds the output projection
    directly (lhsT = h2^T), producing node-major [dst, 64] tiles.
"""

import os
import sys

sys.path.insert(0, "/opt/trn_rl_repo")

import numpy as np

N = int(os.environ.get("GCN_N", 50000))
C_IN = 128
HID = 128
C_OUT = 64
NCORES = 8
NPER = N // NCORES
P = 128
NT = (NPER + P - 1) // P
SPLIT = int(os.environ.get("GCN_SPLIT", 25000))  # table split (int16 idx limit, balanced groups)

MAXIDX = 1024  # max idxs per dma_gather call (larger calls fault the device)

LAST_RESULT = None  # BassKernelResults of the most recent run (for test.py)


def _r16(n):
    return (int(n) + 15) // 16 * 16


def _preprocess(edge_index, x, W1, b1, W2, b2, Wo, bo):
    """Host-side graph preprocessing -> per-core input arrays + schedule."""
    src_e = np.asarray(edge_index[0], np.int64)
    dst_e = np.asarray(edge_index[1], np.int64)
    # degree includes the self loop
    deg = (np.bincount(dst_e, minlength=N) + 1).astype(np.float32)
    disqrt = (1.0 / np.sqrt(deg)).astype(np.float32)

    # gather table: x pre-scaled by src-side normalization
    xs = (np.asarray(x, np.float32) * disqrt[:, None]).astype(np.float16)

    # per (core, tile) edge buckets (no self loops)
    per_core = []
    nlo = np.zeros((NCORES, NT), np.int64)
    nhi = np.zeros((NCORES, NT), np.int64)
    for c in range(NCORES):
        m = (dst_e >= c * NPER) & (dst_e < (c + 1) * NPER)
        s_c = src_e[m]
        d_c = dst_e[m] - c * NPER
        order = np.argsort(d_c, kind="stable")
        s_c, d_c = s_c[order], d_c[order]
        bounds = np.searchsorted(d_c, np.arange(0, NT + 1) * P)
        tiles = []
        for t in range(NT):
            ss = s_c[bounds[t]:bounds[t + 1]]
            dd = d_c[bounds[t]:bounds[t + 1]] - t * P
            lo = ss < SPLIT
            tiles.append((ss[lo], dd[lo], ss[~lo], dd[~lo]))
            nlo[c, t] = lo.sum()
            nhi[c, t] = (~lo).sum()
        per_core.append(tiles)

    # padded idx counts per tile/group (uniform across cores), 16-granular
    M_lo = [_r16(nlo[:, t].max()) for t in range(NT)]
    M_hi = [_r16(nhi[:, t].max()) for t in range(NT)]
    # chunk counts (128-edge blocks fed to matmuls)
    B_lo = [(m + P - 1) // P for m in M_lo]
    B_hi = [(m + P - 1) // P for m in M_hi]
    NB = int(sum(B_lo) + sum(B_hi))
    NC16 = int(sum(M_lo) + sum(M_hi)) // 16  # idx columns (16 idx each)
    tile_ws = [min(P, NPER - t * P) for t in range(NT)]

    in_maps = []
    for c in range(NCORES):
        idx16 = np.zeros((16, NC16), np.int16)
        dstloc = np.zeros((P, NB), np.float32)
        dscale = np.zeros((P, NB), np.float32)
        col16 = 0
        blk = 0
        for t in range(NT):
            ss_lo, dd_lo, ss_hi, dd_hi = per_core[c][t]
            for (ss, dd, m_pad, base) in (
                (ss_lo, dd_lo, M_lo[t], 0),
                (ss_hi, dd_hi, M_hi[t], SPLIT),
            ):
                nb = (m_pad + P - 1) // P
                if nb == 0:
                    continue
                n = len(ss)
                flat_i = np.zeros(m_pad, np.int16)
                flat_i[:n] = (ss - base).astype(np.int16)
                idx16[:, col16:col16 + m_pad // 16] = \
                    flat_i.reshape(m_pad // 16, 16).T
                col16 += m_pad // 16
                flat_d = np.zeros(nb * P, np.float32)
                flat_s = np.zeros(nb * P, np.float32)
                flat_d[:n] = -dd.astype(np.float32)
                flat_s[:n] = disqrt[dd + t * P + c * NPER]
                dstloc[:, blk:blk + nb] = flat_d.reshape(nb, P).T
                dscale[:, blk:blk + nb] = flat_s.reshape(nb, P).T
                blk += nb
        assert col16 == NC16 and blk == NB
        idx_full = np.tile(idx16, (8, 1)).astype(np.int16)

        dsqnm = np.zeros((P, NT), np.float32)
        for t in range(NT):
            tw = tile_ws[t]
            dsqnm[:tw, t] = disqrt[c * NPER + t * P: c * NPER + t * P + tw]

        iota = np.tile(np.arange(P, dtype=np.float16)[None, :], (P, 1))

        in_maps.append({
            "xs": xs,
            "xss": xs[c * NPER:(c + 1) * NPER].copy(),
            "idx": idx_full,
            "dstloc": dstloc,
            "dscale": dscale,
            "ndscale": -dscale,
            "dsqnm": dsqnm,
            "iota": iota,
            "w1": np.asarray(W1, np.float32).astype(np.float16),
            "w2": np.asarray(W2, np.float32).astype(np.float16),
            "wo": np.asarray(Wo, np.float32).astype(np.float16),
            "b1": np.asarray(b1, np.float32).reshape(HID, 1).copy(),
            "b2": np.asarray(b2, np.float32).reshape(HID, 1).copy(),
            "bo": np.tile(np.asarray(bo, np.float32)[None, :], (P, 1)),
        })

    sched = dict(M_lo=M_lo, M_hi=M_hi, B_lo=B_lo, B_hi=B_hi, NB=NB,
                 NC16=NC16, tile_ws=tile_ws)
    return in_maps, sched


def _build_program(sched):
    import concourse.bass as bass
    import concourse.bacc as bacc
    import concourse.tile as tile
    import concourse.mybir as mybir
    from concourse.masks import make_identity

    f32 = mybir.dt.float32
    f16 = mybir.dt.float16
    i16 = mybir.dt.int16
    M_lo, M_hi = sched["M_lo"], sched["M_hi"]
    B_lo, B_hi = sched["B_lo"], sched["B_hi"]
    NB, NC16, tile_ws = sched["NB"], sched["NC16"], sched["tile_ws"]
    nblk_max = max(bl + bh for bl, bh in zip(B_lo, B_hi))

    nc = bacc.Bacc("TRN2", target_bir_lowering=False, debug=False,
                   num_devices=NCORES)

    xs_d = nc.dram_tensor("xs", [N, C_IN], f16, kind="ExternalInput")
    xss_d = nc.dram_tensor("xss", [NPER, C_IN], f16, kind="ExternalInput")
    idx_d = nc.dram_tensor("idx", [P, NC16], i16, kind="ExternalInput")
    dstloc_d = nc.dram_tensor("dstloc", [P, NB], f32, kind="ExternalInput")
    dscale_d = nc.dram_tensor("dscale", [P, NB], f32, kind="ExternalInput")
    ndscale_d = nc.dram_tensor("ndscale", [P, NB], f32, kind="ExternalInput")
    dsqnm_d = nc.dram_tensor("dsqnm", [P, NT], f32, kind="ExternalInput")
    iota_d = nc.dram_tensor("iota", [P, P], f16, kind="ExternalInput")
    w1_d = nc.dram_tensor("w1", [C_IN, HID], f16, kind="ExternalInput")
    w2_d = nc.dram_tensor("w2", [HID, HID], f16, kind="ExternalInput")
    wo_d = nc.dram_tensor("wo", [HID, C_OUT], f16, kind="ExternalInput")
    b1_d = nc.dram_tensor("b1", [HID, 1], f32, kind="ExternalInput")
    b2_d = nc.dram_tensor("b2", [HID, 1], f32, kind="ExternalInput")
    bo_d = nc.dram_tensor("bo", [P, C_OUT], f32, kind="ExternalInput")
    out_d = nc.dram_tensor("out", [NPER, C_OUT], f32, kind="ExternalOutput")

    with tile.TileContext(nc) as tc:
        with tc.tile_pool(name="const", bufs=1) as cpool, \
             tc.tile_pool(name="gather", bufs=4) as gpool, \
             tc.tile_pool(name="smat", bufs=10) as spool, \
             tc.tile_pool(name="work", bufs=3) as wpool, \
             tc.tile_pool(name="psA", bufs=3, space="PSUM") as psA, \
             tc.tile_pool(name="psH", bufs=2, space="PSUM") as psH, \
             tc.tile_pool(name="psT", bufs=2, space="PSUM") as psT, \
             tc.tile_pool(name="dram", bufs=1, space="DRAM") as dram:

            def cload(name, dram_t, shape, dt):
                t = cpool.tile(shape, dt, name=name)
                nc.sync.dma_start(t[:], dram_t[tuple(slice(0, s) for s in shape)])
                return t

            idx_sb = cload("idx_sb", idx_d, [P, NC16], i16)
            dstloc_sb = cload("dstloc_sb", dstloc_d, [P, NB], f32)
            dscale_sb = cload("dscale_sb", dscale_d, [P, NB], f32)
            ndscale_sb = cload("ndscale_sb", ndscale_d, [P, NB], f32)
            dsqnm_sb = cload("dsqnm_sb", dsqnm_d, [P, NT], f32)
            iota_sb = cload("iota_sb", iota_d, [P, P], f16)
            w1_sb = cload("w1_sb", w1_d, [C_IN, HID], f16)
            w2_sb = cload("w2_sb", w2_d, [HID, HID], f16)
            wo_sb = cload("wo_sb", wo_d, [HID, C_OUT], f16)
            b1_sb = cload("b1_sb", b1_d, [HID, 1], f32)
            b2_sb = cload("b2_sb", b2_d, [HID, 1], f32)
            bo_sb = cload("bo_sb", bo_d, [P, C_OUT], f32)

            ident_sb = cpool.tile([P, P], f16, name="ident_sb")
            make_identity(nc, ident_sb[:])

            h1s = dram.tile([NPER, HID], f16, name="h1s")
            h1f = dram.tile([N, HID], f16, name="h1f", addr_space="Shared")

            # register cache for num_idxs_reg values
            regs = {}

            def reg_of(v):
                if v not in regs:
                    regs[v] = nc.gpsimd.to_reg(v)
                return regs[v]

            def layer(phase):
                w_sb = w1_sb if phase == 0 else w2_sb
                b_sb = b1_sb if phase == 0 else b2_sb
                tbl = xs_d if phase == 0 else h1f
                shard = xss_d if phase == 0 else h1s
                col16 = 0
                blk = 0
                for t in range(NT):
                    tw = tile_ws[t]
                    blo, bhi = B_lo[t], B_hi[t]
                    nblk = blo + bhi
                    G = gpool.tile([P, nblk_max, C_IN], f16, tag="G", name="G")
                    for (m_pad, goff, base) in ((M_lo[t], 0, 0),
                                                (M_hi[t], blo, SPLIT)):
                        if m_pad == 0:
                            continue
                        src = tbl[base:, :] if base else tbl[:, :]
                        for o in range(0, m_pad, MAXIDX):
                            n_call = min(MAXIDX, m_pad - o)
                            c0 = col16 + o // 16
                            ob = goff + o // P
                            nc.gpsimd.dma_gather(
                                out_ap=G[:, ob:ob + (n_call + P - 1) // P, :],
                                in_ap=src,
                                idxs_ap=idx_sb[:, c0:c0 + (n_call + 15) // 16],
                                num_idxs=n_call,
                                num_idxs_reg=reg_of(n_call),
                                elem_size=C_IN)
                        col16 += m_pad // 16
                    pa = psA.tile([P, tw], f32, tag="pa", name="pa")
                    # self-loop contribution: contiguous slab + diagonal matmul
                    slab = wpool.tile([P, C_IN], f16, tag="slab", name="slab")
                    nc.sync.dma_start(slab[:tw, :],
                                      shard[t * P:t * P + tw, :])
                    diag = spool.tile([P, P], f16, tag="S", name="diag")
                    nc.scalar.activation(diag[:, :tw], ident_sb[:, :tw],
                                         mybir.ActivationFunctionType.Copy,
                                         scale=dsqnm_sb[:, t:t + 1])
                    nc.tensor.matmul(pa[:], lhsT=slab[:tw, :],
                                     rhs=diag[:tw, :tw], start=True,
                                     stop=False)
                    # valid contraction rows per block (final block of each
                    # group is 16-granular; stale tail rows are never read)
                    ks = []
                    for (m_pad, nb) in ((M_lo[t], blo), (M_hi[t], bhi)):
                        if nb:
                            ks += [P] * (nb - 1) + [m_pad - (nb - 1) * P]
                    for j in range(nblk):
                        # S on ScalarE (keeps DVE off the shared GpSimd SBUF
                        # port): S = relu(dscale - dscale*(iota - dstloc)^2)
                        S1 = spool.tile([P, P], f16, tag="S", name="S1")
                        nc.scalar.activation(
                            S1[:, :tw], iota_sb[:, :tw],
                            mybir.ActivationFunctionType.Square,
                            bias=dstloc_sb[:, blk + j:blk + j + 1])
                        S = spool.tile([P, P], f16, tag="S", name="S")
                        nc.scalar.activation(
                            S[:, :tw], S1[:, :tw],
                            mybir.ActivationFunctionType.Relu,
                            scale=ndscale_sb[:, blk + j:blk + j + 1],
                            bias=dscale_sb[:, blk + j:blk + j + 1])
                        nc.tensor.matmul(pa[:], lhsT=G[:ks[j], j, :],
                                         rhs=S[:ks[j], :tw],
                                         start=False, stop=(j == nblk - 1))
                    blk += nblk
                    agg = wpool.tile([P, tw], f16, tag="agg", name="agg")
                    nc.vector.tensor_copy(agg[:], pa[:])
                    ph = psH.tile([P, tw], f32, tag="ph", name="ph")
                    nc.tensor.matmul(ph[:], lhsT=w_sb[:], rhs=agg[:],
                                     start=True, stop=True)
                    h = wpool.tile([P, tw], f16, tag="h", name="h")
                    nc.scalar.activation(h[:], ph[:],
                                         mybir.ActivationFunctionType.Relu,
                                         bias=b_sb[:, 0:1])
                    if phase == 0:
                        pt = psT.tile([P, P], f16, tag="pt", name="pt")
                        nc.tensor.transpose(out=pt[:tw, :], in_=h[:, :tw],
                                            identity=ident_sb[:])
                        hn = wpool.tile([P, P], f16, tag="hn", name="hn")
                        nc.vector.tensor_scalar(
                            out=hn[:tw, :], in0=pt[:tw, :],
                            scalar1=dsqnm_sb[:tw, t:t + 1], scalar2=None,
                            op0=mybir.AluOpType.mult)
                        nc.sync.dma_start(h1s[t * P:t * P + tw, :], hn[:tw, :])
                    else:
                        po = psT.tile([P, C_OUT], f32, tag="pt", name="po")
                        nc.tensor.matmul(po[:tw, :], lhsT=h[:, :tw],
                                         rhs=wo_sb[:], start=True, stop=True)
                        ob = wpool.tile([P, C_OUT], f32, tag="ob", name="ob")
                        nc.vector.tensor_tensor(out=ob[:tw, :], in0=po[:tw, :],
                                                in1=bo_sb[:tw, :],
                                                op=mybir.AluOpType.add)
                        nc.sync.dma_start(out_d[t * P:t * P + tw, :],
                                          ob[:tw, :])

            layer(0)
            nc.gpsimd.collective_compute(
                "AllGather", mybir.AluOpType.bypass,
                replica_groups=[list(range(NCORES))],
                ins=[h1s[:].opt()], outs=[h1f[:].opt()])
            layer(1)

    nc.compile()
    return nc


def kernel(x, edge_index, W1, b1, W2, b2, Wo, bo):
    global LAST_RESULT
    from concourse import bass_utils

    in_maps, sched = _preprocess(edge_index, x, W1, b1, W2, b2, Wo, bo)
    nc = _build_program(sched)
    res = bass_utils.run_bass_kernel_spmd(nc, in_maps,
                                          core_ids=list(range(NCORES)))
    LAST_RESULT = res
    out = np.concatenate([res.results[c]["out"] for c in range(NCORES)], axis=0)
    return out.astype(np.float32)
